# revision 32
# baseline (speedup 1.0000x reference)
"""MoE-with-DeepGEMM kernel for 8 Trainium2 NeuronCores.

Problem: M=4096 tokens, D=2048 in-dim, H=2048 out-dim, E=8 experts.
    gate = softmax(x @ gate_w.T + gate_b)            # [M, E], fp32
    y    = (q8(x) @ q8(expert_w[e]).T) -> bf16       # [E, M, H]
    out  = sum_e gate[:, e, None] * y[e].astype(f32) # [M, H]

Strategy: data-parallel over tokens (M). Each of the 8 cores gets
M/8 = 512 tokens, all 8 experts' weights, and computes its output slice
independently — no collectives; the host concatenates the slices.

Per-core device work:
  - gating matmul in fp16 ([e,m] logits via gate_w^T stationary), PE
    transpose to [m-partition, e-free], softmax on DVE/ACT,
  - main GEMM in fp8 (e4m3) with perf_mode=DoubleRow (256-deep
    contraction per matmul), accumulating in PSUM f32,
  - PSUM -> bf16 (matches the reference's bf16 cast of y) on ACT,
  - acc += gate * y_bf16 fused on DVE (scalar_tensor_tensor); the last
    expert reads PSUM directly to shorten the kernel tail.

DMA schedule: gating inputs stream first (concurrent pieces; one queue
tops out ~260 GB/s so big tensors are split), then x_q8 and expert-0
weights with explicit deps so the PE starts as data lands; per-expert
4 MB weight loads are double-buffered behind the 28 us/expert compute.

Host-side prep (not device work): fp8 quantize (identical RNE cast the
reference performs), transposes so the contraction dim lands on SBUF
partitions, and the final concat of per-core outputs.
"""

import numpy as np
import ml_dtypes

import concourse.bacc as bacc
import concourse.bass as bass
import concourse.mybir as mybir
import concourse.tile as tile
from concourse import masks
from concourse.tile import add_dep_helper
from concourse.bass_utils import run_bass_kernel_spmd

M, D, H, E = 4096, 2048, 2048, 8
NCORES = 8
MS = M // NCORES          # tokens per core (512)
MC = MS // 128            # m-chunks of 128 partitions (4)
DS = D // 128             # d-subtiles of 128 (16)
KP = DS // 2              # DoubleRow d-pairs of 256 (8)
NH = 512                  # h columns per matmul (one PSUM bank of f32)
HC = H // NH              # h-chunks (4)

_NC = None


def _build_program() -> bass.Bass:
    dt = mybir.dt
    nc = bacc.Bacc(None, target_bir_lowering=False)

    xq = nc.dram_tensor("xq", [D, MS], dt.float8e4, kind="ExternalInput")
    xf = nc.dram_tensor("xf", [D, MS], dt.float16, kind="ExternalInput")
    wq = nc.dram_tensor("wq", [E * D, H], dt.float8e4, kind="ExternalInput")
    gwt = nc.dram_tensor("gwt", [D, E], dt.float16, kind="ExternalInput")
    gb = nc.dram_tensor("gb", [E, 1], dt.float32, kind="ExternalInput")
    out = nc.dram_tensor("out", [MS, H], dt.float32, kind="ExternalOutput")

    with tile.TileContext(nc) as tc, \
            tc.tile_pool(name="const", bufs=1) as constp, \
            tc.tile_pool(name="wpool", bufs=2) as wpool, \
            tc.tile_pool(name="ypool", bufs=22) as ypool, \
            tc.tile_pool(name="small", bufs=8) as small, \
            tc.tile_pool(name="ps", bufs=8, space="PSUM") as psp:

        # Persistent SBUF tensors. Contraction index d = s*128 + p.
        xq_sb = constp.tile([128, DS, MS], dt.float8e4, tag="xq")
        xf_sb = constp.tile([128, DS, MS], dt.float16, tag="xf")
        gwt_sb = constp.tile([128, DS, E], dt.float16, tag="gwt")
        gb_sb = constp.tile([E, 1], dt.float32, tag="gb")
        id8_sb = constp.tile([E, E], dt.float32, tag="id8")
        gate_sb = constp.tile([128, MC * E], dt.float32, tag="gate")
        lg_sb = constp.tile([E, MS], dt.float32, tag="lg")
        acc_sb = constp.tile([128, MC * H], dt.float32, tag="acc")

        masks.make_identity(nc, id8_sb[:])

        # Startup DMAs: x_q8 and expert-0 weight pieces stream FIRST (the
        # main GEMM consumes them as they arrive); the gating inputs (xf)
        # follow after w0, covered by expert-0's second compute phase.
        nc.sync.dma_start(gwt_sb[:], gwt[:, :].rearrange("(s p) e -> p s e", p=128))
        nc.sync.dma_start(gb_sb[:], gb[:, :])
        d_xqs = []
        for j in range(2):
            rsl = slice(j * (D // 2), (j + 1) * (D // 2))
            d = nc.sync.dma_start(
                xq_sb[:, j * (DS // 2):(j + 1) * (DS // 2), :],
                xq[rsl, :].rearrange("(s p) m -> p s m", p=128),
            )
            d_xqs.append(d)

        # ---- Gating emission (called at the e0/e1 boundary) ----
        def emit_gating():
            ps_gt = psp.tile([E, MS], dt.float32, tag="ps", name="ps_gt")
            for s in range(DS):
                nc.tensor.matmul(
                    ps_gt[:],
                    lhsT=gwt_sb[:, s:s + 1, :],
                    rhs=xf_sb[:, s:s + 1, :],
                    start=(s == 0),
                    stop=(s == DS - 1),
                )
            nc.vector.tensor_scalar_add(lg_sb[:], ps_gt[:], gb_sb[:])
            for mc in range(MC):
                pst = psp.tile([128, E], dt.float32, tag="ps", name=f"ps_t{mc}")
                nc.tensor.transpose(
                    pst[:], lg_sb[:, mc * 128:(mc + 1) * 128], id8_sb[:]
                )
                mx = small.tile([128, 1], dt.float32, tag="sm1")
                nc.vector.tensor_reduce(
                    mx[:], pst[:], mybir.AxisListType.X, mybir.AluOpType.max
                )
                nmx = small.tile([128, 1], dt.float32, tag="sm1")
                nc.vector.tensor_scalar_mul(nmx[:], mx[:], -1.0)
                ex = small.tile([128, E], dt.float32, tag="sm")
                ssum = small.tile([128, 1], dt.float32, tag="sm1")
                nc.scalar.activation(
                    ex[:], pst[:], mybir.ActivationFunctionType.Exp,
                    bias=nmx[:], scale=1.0, accum_out=ssum[:],
                )
                rcp = small.tile([128, 1], dt.float32, tag="sm1")
                nc.vector.reciprocal(rcp[:], ssum[:])
                nc.vector.tensor_scalar_mul(
                    gate_sb[:, mc * E:(mc + 1) * E], ex[:], rcp[:]
                )

        # ---- Main GEMM + weighted combine ----
        # Expert 0 runs as two mc-pair phases (k-major, 8 PSUM banks each)
        # so its matmuls consume the w0 pieces at the rate they arrive from
        # HBM; the gate-dependent accumulates are deferred until gating has
        # run at the e0/e1 boundary (where it covers w1's arrival).
        dep = mybir.AluOpType
        for e in range(E):
            w_sb = wpool.tile([128, DS, H], dt.float8e4, tag="w")
            if e == 0:
                w0p = []
                for j in range(4):
                    rsl = slice(j * (D // 4), (j + 1) * (D // 4))
                    dj = nc.sync.dma_start(
                        w_sb[:, j * (DS // 4):(j + 1) * (DS // 4), :],
                        wq[rsl, :].rearrange("(s p) h -> p s h", p=128),
                    )
                    if j > 0:
                        # serial chain: the earliest-needed piece gets full
                        # bandwidth instead of fair-sharing with siblings
                        add_dep_helper(dj.ins, w0p[j - 1].ins, reason="w0 chain")
                    w0p.append(dj)
                d_xfs = []
                for j in range(2):
                    rsl = slice(j * (D // 2), (j + 1) * (D // 2))
                    dxf = nc.sync.dma_start(
                        xf_sb[:, j * (DS // 2):(j + 1) * (DS // 2), :],
                        xf[rsl, :].rearrange("(s p) m -> p s m", p=128),
                    )
                    add_dep_helper(dxf.ins, w0p[3].ins, reason="xf after w0")
                    d_xfs.append(dxf)

                y_hold = []
                for phase_mcs in ((0, 1), (2, 3)):
                    pss = {
                        mc: [
                            psp.tile([128, NH], dt.float32, tag="ps",
                                     name=f"ps0_{mc}_{i}")
                            for i in range(HC)
                        ]
                        for mc in phase_mcs
                    }
                    for k in range(KP):
                        for mc in phase_mcs:
                            lhsT = xq_sb[:, 2 * k:2 * k + 2,
                                         mc * 128:(mc + 1) * 128]
                            for hc in range(HC):
                                nc.tensor.matmul(
                                    pss[mc][hc][:],
                                    lhsT=lhsT,
                                    rhs=w_sb[:, 2 * k:2 * k + 2,
                                             hc * NH:(hc + 1) * NH],
                                    start=(k == 0),
                                    stop=(k == KP - 1),
                                    perf_mode=mybir.MatmulPerfMode.DoubleRow,
                                )
                    for mc in phase_mcs:
                        for hc in range(HC):
                            y = ypool.tile([128, NH], dt.bfloat16, tag="y")
                            nc.scalar.copy(y[:], pss[mc][hc][:])
                            y_hold.append((mc, hc, y))
                emit_gating()
                for mc, hc, y in y_hold:
                    a_ap = acc_sb[:, mc * H + hc * NH:mc * H + (hc + 1) * NH]
                    nc.vector.tensor_scalar_mul(
                        a_ap, y[:], gate_sb[:, mc * E:mc * E + 1]
                    )
                continue
            for j in range(2):
                rsl = slice(e * D + j * (D // 2), e * D + (j + 1) * (D // 2))
                dw = nc.sync.dma_start(
                    w_sb[:, j * (DS // 2):(j + 1) * (DS // 2), :],
                    wq[rsl, :].rearrange("(s p) h -> p s h", p=128),
                )
                if e == 1:
                    add_dep_helper(dw.ins, d_xfs[j].ins, reason="w1 after xf")
            for mc in range(MC):
                msl = slice(mc * 128, (mc + 1) * 128)
                pss = [
                    psp.tile([128, NH], dt.float32, tag="ps", name=f"ps_{e}_{mc}_{i}")
                    for i in range(HC)
                ]
                for k in range(KP):
                    lhsT = xq_sb[:, 2 * k:2 * k + 2, msl]
                    for hc in range(HC):
                        nc.tensor.matmul(
                            pss[hc][:],
                            lhsT=lhsT,
                            rhs=w_sb[:, 2 * k:2 * k + 2, hc * NH:(hc + 1) * NH],
                            start=(k == 0),
                            stop=(k == KP - 1),
                            perf_mode=mybir.MatmulPerfMode.DoubleRow,
                        )
                g_ap = gate_sb[:, mc * E + e:mc * E + e + 1]
                for hc in range(HC):
                    a_ap = acc_sb[:, mc * H + hc * NH:mc * H + (hc + 1) * NH]
                    if e == E - 1:
                        nc.vector.scalar_tensor_tensor(
                            a_ap, pss[hc][:], g_ap, a_ap,
                            op0=mybir.AluOpType.mult, op1=mybir.AluOpType.add,
                        )
                    else:
                        y = ypool.tile([128, NH], dt.bfloat16, tag="y")
                        nc.scalar.copy(y[:], pss[hc][:])
                        nc.vector.scalar_tensor_tensor(
                            a_ap, y[:], g_ap, a_ap,
                            op0=mybir.AluOpType.mult, op1=mybir.AluOpType.add,
                        )
                if e == E - 1:
                    for j in range(4):
                        nc.sync.dma_start(
                            out[mc * 128:(mc + 1) * 128, j * NH:(j + 1) * NH],
                            acc_sb[:, mc * H + j * NH:mc * H + (j + 1) * NH],
                        )

    nc.compile()
    return nc


def _get_nc() -> bass.Bass:
    global _NC
    if _NC is None:
        _NC = _build_program()
    return _NC


def _prep_in_maps(x, gate_w, gate_b, expert_w):
    f8fn = ml_dtypes.float8_e4m3fn
    f8trn = ml_dtypes.float8_e4m3  # same bits as e4m3fn for |v| <= 240

    x = np.asarray(x, dtype=np.float32)
    gate_w = np.asarray(gate_w, dtype=np.float32)
    gate_b = np.asarray(gate_b, dtype=np.float32)
    expert_w = np.asarray(expert_w, dtype=np.float32)

    # x^T: [D, M]; quantized and bf16 (gating) copies.
    xT = np.ascontiguousarray(x.T)                       # [D, M] f32
    xT_bf = xT.astype(np.float16)                        # [D, M] fp16 (gating)
    xqT = xT.astype(f8fn).view(f8trn)                    # [D, M] fp8
    # expert_w [E, H, D] -> w^T per expert [E, D, H], quantized, stacked.
    wqT = np.ascontiguousarray(
        expert_w.transpose(0, 2, 1)
    ).astype(f8fn).view(f8trn).reshape(E * D, H)
    gwt = np.ascontiguousarray(gate_w.T).astype(np.float16)  # [D, E] fp16
    gbb = np.ascontiguousarray(gate_b.reshape(E, 1))

    in_maps = []
    for c in range(NCORES):
        csl = slice(c * MS, (c + 1) * MS)
        in_maps.append({
            "xq": np.ascontiguousarray(xqT[:, csl]),
            "xf": np.ascontiguousarray(xT_bf[:, csl]),
            "wq": wqT,
            "gwt": gwt,
            "gb": gbb,
        })
    return in_maps


def kernel(x, gate_w, gate_b, expert_w, _trace=False, _trace_kwargs=None):
    nc = _get_nc()
    in_maps = _prep_in_maps(x, gate_w, gate_b, expert_w)
    kw = {}
    if _trace:
        kw["trace"] = True
        kw.update(_trace_kwargs or {})
    res = run_bass_kernel_spmd(nc, in_maps, core_ids=list(range(NCORES)), **kw)
    outp = np.concatenate(
        [np.asarray(res.results[c]["out"]) for c in range(NCORES)], axis=0
    )
    if _trace:
        return outp, res
    return outp


# revision 33
# speedup vs baseline: 1.0158x; 1.0158x over previous
"""MoE-with-DeepGEMM kernel for 8 Trainium2 NeuronCores.

Problem: M=4096 tokens, D=2048 in-dim, H=2048 out-dim, E=8 experts.
    gate = softmax(x @ gate_w.T + gate_b)            # [M, E], fp32
    y    = (q8(x) @ q8(expert_w[e]).T) -> bf16       # [E, M, H]
    out  = sum_e gate[:, e, None] * y[e].astype(f32) # [M, H]

Strategy: data-parallel over tokens (M). Each of the 8 cores gets
M/8 = 512 tokens, all 8 experts' weights, and computes its output slice
independently — no collectives; the host concatenates the slices.

Per-core device work:
  - gating matmul in fp16 ([e,m] logits via gate_w^T stationary), PE
    transpose to [m-partition, e-free], softmax on DVE/ACT,
  - main GEMM in fp8 (e4m3) with perf_mode=DoubleRow (256-deep
    contraction per matmul), accumulating in PSUM f32,
  - PSUM -> bf16 (matches the reference's bf16 cast of y) on ACT,
  - acc += gate * y_bf16 fused on DVE (scalar_tensor_tensor); the last
    expert reads PSUM directly to shorten the kernel tail.

DMA schedule: gating inputs stream first (concurrent pieces; one queue
tops out ~260 GB/s so big tensors are split), then x_q8 and expert-0
weights with explicit deps so the PE starts as data lands; per-expert
4 MB weight loads are double-buffered behind the 28 us/expert compute.

Host-side prep (not device work): fp8 quantize (identical RNE cast the
reference performs), transposes so the contraction dim lands on SBUF
partitions, and the final concat of per-core outputs.
"""

import numpy as np
import ml_dtypes

import concourse.bacc as bacc
import concourse.bass as bass
import concourse.mybir as mybir
import concourse.tile as tile
from concourse import masks
from concourse.tile import add_dep_helper
from concourse.bass_utils import run_bass_kernel_spmd

M, D, H, E = 4096, 2048, 2048, 8
NCORES = 8
MS = M // NCORES          # tokens per core (512)
MC = MS // 128            # m-chunks of 128 partitions (4)
DS = D // 128             # d-subtiles of 128 (16)
KP = DS // 2              # DoubleRow d-pairs of 256 (8)
NH = 512                  # h columns per matmul (one PSUM bank of f32)
HC = H // NH              # h-chunks (4)

_NC = None


def _build_program() -> bass.Bass:
    dt = mybir.dt
    nc = bacc.Bacc(None, target_bir_lowering=False)

    xq = nc.dram_tensor("xq", [D, MS], dt.float8e4, kind="ExternalInput")
    xf = nc.dram_tensor("xf", [D, MS], dt.float16, kind="ExternalInput")
    wq = nc.dram_tensor("wq", [E * D, H], dt.float8e4, kind="ExternalInput")
    gwt = nc.dram_tensor("gwt", [D, E], dt.float16, kind="ExternalInput")
    gb = nc.dram_tensor("gb", [E, 1], dt.float32, kind="ExternalInput")
    out = nc.dram_tensor("out", [MS, H], dt.float32, kind="ExternalOutput")

    with tile.TileContext(nc) as tc, \
            tc.tile_pool(name="const", bufs=1) as constp, \
            tc.tile_pool(name="wpool", bufs=2) as wpool, \
            tc.tile_pool(name="ypool", bufs=22) as ypool, \
            tc.tile_pool(name="small", bufs=8) as small, \
            tc.tile_pool(name="ps", bufs=8, space="PSUM") as psp:

        # Persistent SBUF tensors. Contraction index d = s*128 + p.
        xq_sb = constp.tile([128, DS, MS], dt.float8e4, tag="xq")
        xf_sb = constp.tile([128, DS, MS], dt.float16, tag="xf")
        gwt_sb = constp.tile([128, DS, E], dt.float16, tag="gwt")
        gb_sb = constp.tile([E, 1], dt.float32, tag="gb")
        id8_sb = constp.tile([E, E], dt.float32, tag="id8")
        gate_sb = constp.tile([128, MC * E], dt.float32, tag="gate")
        lg_sb = constp.tile([E, MS], dt.float32, tag="lg")
        acc_sb = constp.tile([128, MC * H], dt.float32, tag="acc")

        masks.make_identity(nc, id8_sb[:])

        # Startup DMAs: x_q8 and expert-0 weight pieces stream FIRST (the
        # main GEMM consumes them as they arrive); the gating inputs (xf)
        # follow after w0, covered by expert-0's second compute phase.
        nc.sync.dma_start(gwt_sb[:], gwt[:, :].rearrange("(s p) e -> p s e", p=128))
        nc.sync.dma_start(gb_sb[:], gb[:, :])
        d_xqs = []
        for j in range(2):
            rsl = slice(j * (D // 2), (j + 1) * (D // 2))
            d = nc.sync.dma_start(
                xq_sb[:, j * (DS // 2):(j + 1) * (DS // 2), :],
                xq[rsl, :].rearrange("(s p) m -> p s m", p=128),
            )
            d_xqs.append(d)

        # ---- Gating emission (called at the e0/e1 boundary) ----
        def emit_gating():
            ps_gt = psp.tile([E, MS], dt.float32, tag="ps", name="ps_gt")
            for s in range(DS):
                nc.tensor.matmul(
                    ps_gt[:],
                    lhsT=gwt_sb[:, s:s + 1, :],
                    rhs=xf_sb[:, s:s + 1, :],
                    start=(s == 0),
                    stop=(s == DS - 1),
                )
            nc.vector.tensor_scalar_add(lg_sb[:], ps_gt[:], gb_sb[:])
            for mc in range(MC):
                pst = psp.tile([128, E], dt.float32, tag="ps", name=f"ps_t{mc}")
                nc.tensor.transpose(
                    pst[:], lg_sb[:, mc * 128:(mc + 1) * 128], id8_sb[:]
                )
                mx = small.tile([128, 1], dt.float32, tag="sm1")
                nc.vector.tensor_reduce(
                    mx[:], pst[:], mybir.AxisListType.X, mybir.AluOpType.max
                )
                nmx = small.tile([128, 1], dt.float32, tag="sm1")
                nc.vector.tensor_scalar_mul(nmx[:], mx[:], -1.0)
                ex = small.tile([128, E], dt.float32, tag="sm")
                ssum = small.tile([128, 1], dt.float32, tag="sm1")
                nc.scalar.activation(
                    ex[:], pst[:], mybir.ActivationFunctionType.Exp,
                    bias=nmx[:], scale=1.0, accum_out=ssum[:],
                )
                rcp = small.tile([128, 1], dt.float32, tag="sm1")
                nc.vector.reciprocal(rcp[:], ssum[:])
                nc.vector.tensor_scalar_mul(
                    gate_sb[:, mc * E:(mc + 1) * E], ex[:], rcp[:]
                )

        # ---- Main GEMM + weighted combine ----
        # Expert 0 runs as two mc-pair phases (k-major, 8 PSUM banks each)
        # so its matmuls consume the w0 pieces at the rate they arrive from
        # HBM; the gate-dependent accumulates are deferred until gating has
        # run at the e0/e1 boundary (where it covers w1's arrival).
        dep = mybir.AluOpType
        for e in range(E):
            w_sb = wpool.tile([128, DS, H], dt.float8e4, tag="w")
            if e == 0:
                w0p = []
                for j in range(4):
                    rsl = slice(j * (D // 4), (j + 1) * (D // 4))
                    dj = nc.sync.dma_start(
                        w_sb[:, j * (DS // 4):(j + 1) * (DS // 4), :],
                        wq[rsl, :].rearrange("(s p) h -> p s h", p=128),
                    )
                    if j == 1:
                        add_dep_helper(dj.ins, d_xqs[0].ins, reason="w0p1 after xq0")
                    elif j == 2:
                        add_dep_helper(dj.ins, d_xqs[1].ins, reason="w0p2 after xq1")
                    elif j == 3:
                        add_dep_helper(dj.ins, w0p[0].ins, reason="w0p3 after w0p0")
                    w0p.append(dj)
                d_xfs = []
                for j in range(2):
                    rsl = slice(j * (D // 2), (j + 1) * (D // 2))
                    dxf = nc.sync.dma_start(
                        xf_sb[:, j * (DS // 2):(j + 1) * (DS // 2), :],
                        xf[rsl, :].rearrange("(s p) m -> p s m", p=128),
                    )
                    add_dep_helper(dxf.ins, w0p[3].ins, reason="xf after w0")
                    d_xfs.append(dxf)

                y_hold = []
                for phase_mcs in ((0, 1), (2, 3)):
                    pss = {
                        mc: [
                            psp.tile([128, NH], dt.float32, tag="ps",
                                     name=f"ps0_{mc}_{i}")
                            for i in range(HC)
                        ]
                        for mc in phase_mcs
                    }
                    for k in range(KP):
                        for mc in phase_mcs:
                            lhsT = xq_sb[:, 2 * k:2 * k + 2,
                                         mc * 128:(mc + 1) * 128]
                            for hc in range(HC):
                                nc.tensor.matmul(
                                    pss[mc][hc][:],
                                    lhsT=lhsT,
                                    rhs=w_sb[:, 2 * k:2 * k + 2,
                                             hc * NH:(hc + 1) * NH],
                                    start=(k == 0),
                                    stop=(k == KP - 1),
                                    perf_mode=mybir.MatmulPerfMode.DoubleRow,
                                )
                    for mc in phase_mcs:
                        for hc in range(HC):
                            y = ypool.tile([128, NH], dt.bfloat16, tag="y")
                            nc.scalar.copy(y[:], pss[mc][hc][:])
                            y_hold.append((mc, hc, y))
                emit_gating()
                for mc, hc, y in y_hold:
                    a_ap = acc_sb[:, mc * H + hc * NH:mc * H + (hc + 1) * NH]
                    nc.vector.tensor_scalar_mul(
                        a_ap, y[:], gate_sb[:, mc * E:mc * E + 1]
                    )
                continue
            for j in range(2):
                rsl = slice(e * D + j * (D // 2), e * D + (j + 1) * (D // 2))
                dw = nc.sync.dma_start(
                    w_sb[:, j * (DS // 2):(j + 1) * (DS // 2), :],
                    wq[rsl, :].rearrange("(s p) h -> p s h", p=128),
                )
                if e == 1:
                    add_dep_helper(dw.ins, d_xfs[j].ins, reason="w1 after xf")
            for mc in range(MC):
                msl = slice(mc * 128, (mc + 1) * 128)
                pss = [
                    psp.tile([128, NH], dt.float32, tag="ps", name=f"ps_{e}_{mc}_{i}")
                    for i in range(HC)
                ]
                for k in range(KP):
                    lhsT = xq_sb[:, 2 * k:2 * k + 2, msl]
                    for hc in range(HC):
                        nc.tensor.matmul(
                            pss[hc][:],
                            lhsT=lhsT,
                            rhs=w_sb[:, 2 * k:2 * k + 2, hc * NH:(hc + 1) * NH],
                            start=(k == 0),
                            stop=(k == KP - 1),
                            perf_mode=mybir.MatmulPerfMode.DoubleRow,
                        )
                g_ap = gate_sb[:, mc * E + e:mc * E + e + 1]
                for hc in range(HC):
                    a_ap = acc_sb[:, mc * H + hc * NH:mc * H + (hc + 1) * NH]
                    if e == E - 1:
                        nc.vector.scalar_tensor_tensor(
                            a_ap, pss[hc][:], g_ap, a_ap,
                            op0=mybir.AluOpType.mult, op1=mybir.AluOpType.add,
                        )
                    else:
                        y = ypool.tile([128, NH], dt.bfloat16, tag="y")
                        nc.scalar.copy(y[:], pss[hc][:])
                        nc.vector.scalar_tensor_tensor(
                            a_ap, y[:], g_ap, a_ap,
                            op0=mybir.AluOpType.mult, op1=mybir.AluOpType.add,
                        )
                if e == E - 1:
                    for j in range(4):
                        nc.sync.dma_start(
                            out[mc * 128:(mc + 1) * 128, j * NH:(j + 1) * NH],
                            acc_sb[:, mc * H + j * NH:mc * H + (j + 1) * NH],
                        )

    nc.compile()
    return nc


def _get_nc() -> bass.Bass:
    global _NC
    if _NC is None:
        _NC = _build_program()
    return _NC


def _prep_in_maps(x, gate_w, gate_b, expert_w):
    f8fn = ml_dtypes.float8_e4m3fn
    f8trn = ml_dtypes.float8_e4m3  # same bits as e4m3fn for |v| <= 240

    x = np.asarray(x, dtype=np.float32)
    gate_w = np.asarray(gate_w, dtype=np.float32)
    gate_b = np.asarray(gate_b, dtype=np.float32)
    expert_w = np.asarray(expert_w, dtype=np.float32)

    # x^T: [D, M]; quantized and bf16 (gating) copies.
    xT = np.ascontiguousarray(x.T)                       # [D, M] f32
    xT_bf = xT.astype(np.float16)                        # [D, M] fp16 (gating)
    xqT = xT.astype(f8fn).view(f8trn)                    # [D, M] fp8
    # expert_w [E, H, D] -> w^T per expert [E, D, H], quantized, stacked.
    wqT = np.ascontiguousarray(
        expert_w.transpose(0, 2, 1)
    ).astype(f8fn).view(f8trn).reshape(E * D, H)
    gwt = np.ascontiguousarray(gate_w.T).astype(np.float16)  # [D, E] fp16
    gbb = np.ascontiguousarray(gate_b.reshape(E, 1))

    in_maps = []
    for c in range(NCORES):
        csl = slice(c * MS, (c + 1) * MS)
        in_maps.append({
            "xq": np.ascontiguousarray(xqT[:, csl]),
            "xf": np.ascontiguousarray(xT_bf[:, csl]),
            "wq": wqT,
            "gwt": gwt,
            "gb": gbb,
        })
    return in_maps


def kernel(x, gate_w, gate_b, expert_w, _trace=False, _trace_kwargs=None):
    nc = _get_nc()
    in_maps = _prep_in_maps(x, gate_w, gate_b, expert_w)
    kw = {}
    if _trace:
        kw["trace"] = True
        kw.update(_trace_kwargs or {})
    res = run_bass_kernel_spmd(nc, in_maps, core_ids=list(range(NCORES)), **kw)
    outp = np.concatenate(
        [np.asarray(res.results[c]["out"]) for c in range(NCORES)], axis=0
    )
    if _trace:
        return outp, res
    return outp
